# revision 6
# baseline (speedup 1.0000x reference)
"""Trainium2 Bass kernel for retrieval-kNN NeuronMemory module.

Strategy: data-parallel over the 8192 token rows -> 1024 rows per core,
each core independent (no collectives):
  - sc   = (memory_weights[b] . compress_neurons) * 1/sqrt(R)   (DVE weighted sum)
  - Q^T  = sc^T @ x^T  (PE; x tiles transposed on PE)
  - scores = Q @ K^T in 32 chunks of 1024 columns (PE fp32, PSUM resident)
  - per chunk: DVE max8 + max_index8 directly on PSUM -> 256 candidates/row
  - merge: global top-8 of candidates + arithmetic index select (DVE)
  - softmax on ACT/DVE; V rows gathered via indirect DMA; weighted sum on DVE
"""

import numpy as np

import concourse.bass as bass
import concourse.mybir as mybir
from concourse import bacc
from concourse.bass import ts, ds, IndirectOffsetOnAxis
from concourse.tile import TileContext

F32 = mybir.dt.float32
I32 = mybir.dt.int32
U16 = mybir.dt.uint16
OP = mybir.AluOpType
AX = mybir.AxisListType
AF = mybir.ActivationFunctionType

P = 128
BIG = 65536.0


def build_retrieval_kernel(tc, outs, ins, *, rows, d, rdim, nk, ncomp, topk, chunk):
    """Emit the per-core program. ins/outs are dicts of DRAM APs."""
    nc = tc.nc
    RT = rows // P          # row tiles
    DT = d // P             # d tiles
    NCH = nk // chunk       # score chunks
    CAND = NCH * 8          # candidates per row
    KJ = chunk // P         # K transpose tiles per chunk
    NMM = chunk // 512      # matmuls per chunk
    assert chunk % 512 == 0 and nk % chunk == 0 and rows % P == 0 and d % P == 0
    assert rdim == P

    xin = ins["x"]
    mw = ins["mw"]
    cn = ins["cn"]
    kk = ins["kk"]
    kv = ins["kv"]
    ident = ins["ident"]
    coffs = ins["coffs"]
    out_o = outs["out"]
    out_i = outs["idx"]
    out_w = outs["wts"]

    from contextlib import ExitStack

    with ExitStack() as ctx:
        const_pool = ctx.enter_context(tc.tile_pool(name="const", bufs=1))
        sc_pool = ctx.enter_context(tc.tile_pool(name="scp", bufs=1))
        qt_pool = ctx.enter_context(tc.tile_pool(name="qtp", bufs=1))
        cand_pool = ctx.enter_context(tc.tile_pool(name="candp", bufs=1))

        ident_sb = const_pool.tile([P, P], F32, tag="ident")
        nc.sync.dma_start(out=ident_sb, in_=ident[:])
        coffs_sb = const_pool.tile([P, CAND], F32, tag="coffs")
        nc.sync.dma_start(
            out=coffs_sb,
            in_=coffs.rearrange("(o f) -> o f", o=1).to_broadcast([P, CAND]),
        )
        mw_sb = const_pool.tile([P, ncomp], F32, tag="mw")
        nc.sync.dma_start(
            out=mw_sb,
            in_=mw.rearrange("(o f) -> o f", o=1).to_broadcast([P, ncomp]),
        )

        # ---- Phase 0: sc[p, dt, r] = sum_n w_n * cn[n, dt*128+p, r], scaled
        sc_t = sc_pool.tile([P, DT * rdim], F32, tag="sc")
        with tc.tile_pool(name="cnp", bufs=2) as cn_pool:
            for n in range(ncomp):
                cnt = cn_pool.tile([P, DT * rdim], F32, tag="cn")
                nc.sync.dma_start(
                    out=cnt[:].rearrange("p (dt r) -> p dt r", r=rdim),
                    in_=cn[n].rearrange("(dt p) r -> p dt r", p=P),
                )
                wn = mw_sb[:, n : n + 1]
                if n == 0:
                    nc.vector.tensor_scalar(sc_t, cnt, wn, None, OP.mult)
                else:
                    nc.vector.scalar_tensor_tensor(
                        out=sc_t, in0=cnt, scalar=wn, in1=sc_t,
                        op0=OP.mult, op1=OP.add,
                    )
        nc.vector.tensor_scalar(
            sc_t, sc_t, float(1.0 / np.sqrt(rdim)), None, OP.mult
        )

        # ---- Phase A: QT[r, rows] = sc^T @ x^T
        qt = qt_pool.tile([P, rows], F32, tag="qt")
        with (
            tc.tile_pool(name="xsrc", bufs=3) as x_pool,
            tc.tile_pool(name="xT", bufs=3) as xT_pool,
            tc.tile_pool(name="psA_tr", bufs=2, space="PSUM") as psA_tr,
            tc.tile_pool(name="psA_mm", bufs=2, space="PSUM") as psA_mm,
        ):
            for t in range(RT):
                ps_qt = psA_mm.tile([P, P], F32, tag="psqt")
                for dt in range(DT):
                    xsrc = x_pool.tile([P, P], F32, tag="xsrc")
                    nc.sync.dma_start(out=xsrc, in_=xin[ts(t, P), ts(dt, P)])
                    ps_x = psA_tr.tile([P, P], F32, tag="psx")
                    nc.tensor.transpose(out=ps_x, in_=xsrc, identity=ident_sb)
                    xT = xT_pool.tile([P, P], F32, tag="xT")
                    nc.scalar.copy(out=xT, in_=ps_x)
                    nc.tensor.matmul(
                        out=ps_qt,
                        lhsT=sc_t[:, ts(dt, rdim)],
                        rhs=xT,
                        start=(dt == 0),
                        stop=(dt == DT - 1),
                    )
                nc.scalar.copy(out=qt[:, ts(t, P)], in_=ps_qt)

        # ---- Phase B: scores by chunk; per-chunk top-8 vals + local indices
        cand_v = cand_pool.tile([P, RT, CAND], F32, tag="cv")
        cand_i = cand_pool.tile([P, RT, CAND], U16, tag="ci")

        with (
            tc.tile_pool(name="ksrc", bufs=3) as ks_pool,
            tc.tile_pool(name="ktile", bufs=2) as kt_pool,
            tc.tile_pool(name="psB_tr", bufs=2, space="PSUM") as psB_tr,
            tc.tile_pool(name="psB_mm", bufs=2, space="PSUM") as psB_mm,
        ):
            for cc in range(NCH):
                ktile = kt_pool.tile([P, chunk], F32, tag="kt")
                for j in range(KJ):
                    ksrc = ks_pool.tile([P, P], F32, tag="ks")
                    nc.sync.dma_start(
                        out=ksrc, in_=kk[ds(cc * chunk + j * P, P), :]
                    )
                    ps_k = psB_tr.tile([P, P], F32, tag="psk")
                    nc.tensor.transpose(out=ps_k, in_=ksrc, identity=ident_sb)
                    nc.scalar.copy(out=ktile[:, ts(j, P)], in_=ps_k)
                for t in range(RT):
                    ps_s = psB_mm.tile([P, chunk], F32, tag="pss")
                    for m in range(NMM):
                        nc.tensor.matmul(
                            out=ps_s[:, ts(m, 512)],
                            lhsT=qt[:, ts(t, P)],
                            rhs=ktile[:, ts(m, 512)],
                            start=True,
                            stop=True,
                        )
                    nc.vector.max(out=cand_v[:, t, ts(cc, 8)], in_=ps_s)
                    nc.vector.max_index(
                        out=cand_i[:, t, ts(cc, 8)],
                        in_max=cand_v[:, t, ts(cc, 8)],
                        in_values=ps_s,
                    )

        # ---- Phase C: merge, softmax, gather, weighted sum (per row tile)
        with (
            tc.tile_pool(name="mpool", bufs=2) as m_pool,
            tc.tile_pool(name="epool", bufs=2) as e_pool,
            tc.tile_pool(name="spool", bufs=2) as s_pool,
            tc.tile_pool(name="vgp", bufs=2) as vg_pool,
            tc.tile_pool(name="accp", bufs=2) as acc_pool,
        ):
            for t in range(RT):
                candf = m_pool.tile([P, CAND], F32, tag="candf")
                nc.vector.tensor_copy(out=candf, in_=cand_i[:, t, :])
                nc.vector.tensor_tensor(
                    out=candf, in0=candf, in1=coffs_sb, op=OP.add
                )
                fin8 = s_pool.tile([P, 8], F32, tag="fin8")
                nc.vector.max(out=fin8, in_=cand_v[:, t, :])
                # bmg = BIG - global_idx
                bmg = m_pool.tile([P, CAND], F32, tag="bmg")
                nc.vector.tensor_scalar(bmg, candf, -1.0, BIG, OP.mult, OP.add)
                # eq[j, c] = (cand_v[c] == fin8[j])
                eq = e_pool.tile([P, topk, CAND], F32, tag="eq")
                cv_b = (
                    cand_v[:, t, :]
                    .rearrange("p (o c) -> p o c", o=1)
                    .to_broadcast([P, topk, CAND])
                )
                f8_b = (
                    fin8[:]
                    .rearrange("p (k o) -> p k o", o=1)
                    .to_broadcast([P, topk, CAND])
                )
                bmg_b = (
                    bmg[:]
                    .rearrange("p (o c) -> p o c", o=1)
                    .to_broadcast([P, topk, CAND])
                )
                nc.vector.tensor_tensor(out=eq, in0=cv_b, in1=f8_b, op=OP.is_equal)
                nc.vector.tensor_tensor(out=eq, in0=eq, in1=bmg_b, op=OP.mult)
                red = s_pool.tile([P, topk], F32, tag="red")
                nc.vector.tensor_reduce(out=red, in_=eq, axis=AX.X, op=OP.max)
                gidx = s_pool.tile([P, topk], F32, tag="gidx")
                nc.vector.tensor_scalar(gidx, red, -1.0, BIG, OP.mult, OP.add)
                idx32 = s_pool.tile([P, topk], I32, tag="idx32")
                nc.vector.tensor_copy(out=idx32, in_=gidx)
                nc.sync.dma_start(out=out_i[ts(t, P), :], in_=idx32)

                # softmax over fin8 (sorted desc; fin8[:,0] is the max)
                nm = s_pool.tile([P, 1], F32, tag="nm")
                nc.vector.tensor_scalar(nm, fin8[:, 0:1], -1.0, None, OP.mult)
                e8 = s_pool.tile([P, topk], F32, tag="e8")
                ssum = s_pool.tile([P, 1], F32, tag="ssum")
                nc.scalar.activation(
                    out=e8, in_=fin8, func=AF.Exp, bias=nm[:], scale=1.0,
                    accum_out=ssum[:],
                )
                rcp = s_pool.tile([P, 1], F32, tag="rcp")
                nc.vector.reciprocal(rcp, ssum)
                wts = s_pool.tile([P, topk], F32, tag="wts")
                nc.vector.tensor_scalar(wts, e8, rcp[:], None, OP.mult)
                nc.sync.dma_start(out=out_w[ts(t, P), :], in_=wts)

                # gather V rows: vg[p, k, :] = kv[idx32[p, k], :]
                # (HW consumes one offset per dest partition-row -> one
                #  indirect DMA per k)
                vg = vg_pool.tile([P, topk, d], F32, tag="vg")
                for k in range(topk):
                    nc.gpsimd.indirect_dma_start(
                        out=vg[:, k, :],
                        out_offset=None,
                        in_=kv[:],
                        in_offset=IndirectOffsetOnAxis(
                            ap=idx32[:, k : k + 1], axis=0
                        ),
                    )
                # weighted sum over the k gathered rows
                acc = acc_pool.tile([P, d], F32, tag="acc")
                nc.vector.tensor_scalar(acc, vg[:, 0, :], wts[:, 0:1], None, OP.mult)
                for k in range(1, topk):
                    nc.vector.scalar_tensor_tensor(
                        out=acc, in0=vg[:, k, :], scalar=wts[:, k : k + 1],
                        in1=acc, op0=OP.mult, op1=OP.add,
                    )
                nc.sync.dma_start(out=out_o[ts(t, P), :], in_=acc)


# ---------------------------------------------------------------------------
# Host-side entry point: full problem, 8 cores.

B, S, D = 4, 2048, 1024
N_COMPRESS = 16
N_KNOWLEDGE = 32768
RANK = 128
TOPK = 8
N_CORES = 8
ROWS_PER_CORE = (B * S) // N_CORES
CHUNK = 1024


def _build_full():
    nc = bacc.Bacc("TRN2", target_bir_lowering=False, debug=False)
    dts = {}
    dts["x"] = nc.dram_tensor("x", [ROWS_PER_CORE, D], F32, kind="ExternalInput")
    dts["mw"] = nc.dram_tensor("mw", [N_COMPRESS], F32, kind="ExternalInput")
    dts["cn"] = nc.dram_tensor(
        "cn", [N_COMPRESS, D, RANK], F32, kind="ExternalInput"
    )
    dts["kk"] = nc.dram_tensor("kk", [N_KNOWLEDGE, RANK], F32, kind="ExternalInput")
    dts["kv"] = nc.dram_tensor("kv", [N_KNOWLEDGE, D], F32, kind="ExternalInput")
    dts["ident"] = nc.dram_tensor("ident", [P, P], F32, kind="ExternalInput")
    ncand = (N_KNOWLEDGE // CHUNK) * 8
    dts["coffs"] = nc.dram_tensor("coffs", [ncand], F32, kind="ExternalInput")
    outs = {
        "out": nc.dram_tensor(
            "out", [ROWS_PER_CORE, D], F32, kind="ExternalOutput"
        ),
        "idx": nc.dram_tensor(
            "idx", [ROWS_PER_CORE, TOPK], I32, kind="ExternalOutput"
        ),
        "wts": nc.dram_tensor(
            "wts", [ROWS_PER_CORE, TOPK], F32, kind="ExternalOutput"
        ),
    }
    ins_aps = {k: v.ap() for k, v in dts.items()}
    outs_aps = {k: v.ap() for k, v in outs.items()}
    with TileContext(nc) as tc:
        build_retrieval_kernel(
            tc, outs_aps, ins_aps,
            rows=ROWS_PER_CORE, d=D, rdim=RANK, nk=N_KNOWLEDGE,
            ncomp=N_COMPRESS, topk=TOPK, chunk=CHUNK,
        )
    nc.compile()
    return nc


_NC_CACHE = None
LAST_RESULTS = None


def prepare_in_maps(x, memory_weights, compress_neurons, knowledge_K, knowledge_V):
    x = np.ascontiguousarray(np.asarray(x, dtype=np.float32)).reshape(B * S, D)
    mw = np.asarray(memory_weights, dtype=np.float32)
    cn = np.ascontiguousarray(np.asarray(compress_neurons, dtype=np.float32))
    kk = np.ascontiguousarray(np.asarray(knowledge_K, dtype=np.float32))
    kv = np.ascontiguousarray(np.asarray(knowledge_V, dtype=np.float32))

    ident = np.eye(P, dtype=np.float32)
    ncand = (N_KNOWLEDGE // CHUNK) * 8
    coffs = ((np.arange(ncand) // 8) * CHUNK).astype(np.float32)

    in_maps = []
    for c in range(N_CORES):
        in_maps.append(
            {
                "x": x[c * ROWS_PER_CORE : (c + 1) * ROWS_PER_CORE],
                "mw": mw[(c * ROWS_PER_CORE) // S],
                "cn": cn,
                "kk": kk,
                "kv": kv,
                "ident": ident,
                "coffs": coffs,
            }
        )
    return in_maps


def assemble_outputs(outs):
    output = np.concatenate([outs[c]["out"] for c in range(N_CORES)], axis=0)
    topk_idx = np.concatenate([outs[c]["idx"] for c in range(N_CORES)], axis=0)
    weights = np.concatenate([outs[c]["wts"] for c in range(N_CORES)], axis=0)
    output = output.reshape(B, S, D)
    topk_idx = topk_idx.reshape(B, S, TOPK).astype(np.int32)
    weights = weights.reshape(B, S, TOPK)
    return output, topk_idx, weights


def kernel(x, memory_weights, compress_neurons, knowledge_K, knowledge_V, **kwargs):
    from concourse.bass_utils import run_bass_kernel_spmd

    global _NC_CACHE
    if _NC_CACHE is None:
        _NC_CACHE = _build_full()
    nc = _NC_CACHE

    in_maps = prepare_in_maps(
        x, memory_weights, compress_neurons, knowledge_K, knowledge_V
    )
    res = run_bass_kernel_spmd(nc, in_maps, core_ids=list(range(N_CORES)), **kwargs)
    global LAST_RESULTS
    LAST_RESULTS = res
    return assemble_outputs(res.results)


# revision 7
# speedup vs baseline: 1.1811x; 1.1811x over previous
"""Trainium2 Bass kernel for retrieval-kNN NeuronMemory module.

Strategy (data-parallel over the 8192 token rows -> 1024 rows per core, no
collectives):
  - sc   = (memory_weights[b] . compress_neurons) * 1/sqrt(R)   (DVE)
  - Q^T  = sc^T @ x^T in fp32 (PE; x tiles transposed on PE); also kept in
    bf16 for the approximate scores matmul.
  - approx scores = Q_bf16 @ K_bf16^T in 32 chunks of 1024 cols (PE, PSUM)
  - ACT copies each PSUM chunk to SBUF as bf16 into the HIGH halfword of a
    u32 "lattice" whose LOW halfword holds the column index -> each u32,
    viewed as f32, orders like the bf16 score.  One DVE max8 per chunk
    yields 8 (value|index)-packed candidates; 32 chunks -> 256/row.
  - merge: top-16 of the 256 candidates (max8 + match_replace + max8,
    positions via max_index on the 256-wide array -> chunk id).
  - exact rescore: gather the 16 K rows, dot with exact fp32 Q row -> exact
    scores; top-8 of those (max8) gives exactly the reference ranking.
  - softmax on ACT; V rows gathered via indirect DMA; weighted sum on DVE.
"""

import numpy as np

import concourse.bass as bass
import concourse.mybir as mybir
from concourse import bacc
from concourse.bass import ts, ds, IndirectOffsetOnAxis
from concourse.tile import TileContext

F32 = mybir.dt.float32
BF16 = mybir.dt.bfloat16
I32 = mybir.dt.int32
U32 = mybir.dt.uint32
U16 = mybir.dt.uint16
OP = mybir.AluOpType
AX = mybir.AxisListType
AF = mybir.ActivationFunctionType

P = 128
BIG = 65536.0
NCAND = 16  # candidates kept for exact rescoring


def build_retrieval_kernel(tc, outs, ins, *, rows, d, rdim, nk, ncomp, topk, chunk):
    """Emit the per-core program. ins/outs are dicts of DRAM APs."""
    nc = tc.nc
    RT = rows // P          # row tiles
    DT = d // P             # d tiles
    NCH = nk // chunk       # score chunks
    CAND = NCH * 8          # packed candidates per row
    KJ = chunk // P         # K transpose tiles per chunk
    NMM = chunk // 512      # matmuls per chunk
    assert chunk % 512 == 0 and nk % chunk == 0 and rows % P == 0 and d % P == 0
    assert rdim == P and chunk <= 65536

    xin = ins["x"]
    mw = ins["mw"]
    cn = ins["cn"]
    kk = ins["kk"]
    kv = ins["kv"]
    ident = ins["ident"]
    iotalat = ins["iotalat"]
    out_o = outs["out"]
    out_i = outs["idx"]
    out_w = outs["wts"]

    from contextlib import ExitStack

    with ExitStack() as ctx:
        const_pool = ctx.enter_context(tc.tile_pool(name="const", bufs=1))
        sc_pool = ctx.enter_context(tc.tile_pool(name="scp", bufs=1))
        qt_pool = ctx.enter_context(tc.tile_pool(name="qtp", bufs=1))
        cand_pool = ctx.enter_context(tc.tile_pool(name="candp", bufs=1))
        lat_pool = ctx.enter_context(tc.tile_pool(name="latp", bufs=1))

        ident_sb = const_pool.tile([P, P], F32, tag="ident")
        nc.sync.dma_start(out=ident_sb, in_=ident[:])
        mw_sb = const_pool.tile([P, ncomp], F32, tag="mw")
        nc.sync.dma_start(
            out=mw_sb,
            in_=mw.rearrange("(o f) -> o f", o=1).to_broadcast([P, ncomp]),
        )
        # two score lattices; low u16 halfword = column index (persists),
        # high halfword gets the bf16 score per chunk (ACT strided write)
        lats = []
        for li in range(2):
            lt = lat_pool.tile([P, chunk], F32, tag=f"lat{li}")
            nc.sync.dma_start(
                out=lt[:].bitcast(U32),
                in_=iotalat.rearrange("(o f) -> o f", o=1).to_broadcast([P, chunk]),
            )
            lats.append(lt)

        # ---- Phase 0: sc[p, dt, r] = sum_n w_n * cn[n, dt*128+p, r], scaled
        sc_t = sc_pool.tile([P, DT * rdim], F32, tag="sc")
        with tc.tile_pool(name="cnp", bufs=2) as cn_pool:
            for n in range(ncomp):
                cnt = cn_pool.tile([P, DT * rdim], F32, tag="cn")
                nc.sync.dma_start(
                    out=cnt[:].rearrange("p (dt r) -> p dt r", r=rdim),
                    in_=cn[n].rearrange("(dt p) r -> p dt r", p=P),
                )
                wn = mw_sb[:, n : n + 1]
                if n == 0:
                    nc.vector.tensor_scalar(sc_t, cnt, wn, None, OP.mult)
                else:
                    nc.vector.scalar_tensor_tensor(
                        out=sc_t, in0=cnt, scalar=wn, in1=sc_t,
                        op0=OP.mult, op1=OP.add,
                    )
        nc.vector.tensor_scalar(
            sc_t, sc_t, float(1.0 / np.sqrt(rdim)), None, OP.mult
        )

        # ---- Phase A: QT[r, rows] = sc^T @ x^T  (fp32 exact + bf16 copy)
        qt32 = qt_pool.tile([P, rows], F32, tag="qt32")
        qtbf = qt_pool.tile([P, rows], BF16, tag="qtbf")
        with (
            tc.tile_pool(name="xsrc", bufs=3) as x_pool,
            tc.tile_pool(name="xT", bufs=3) as xT_pool,
            tc.tile_pool(name="psA_tr", bufs=2, space="PSUM") as psA_tr,
            tc.tile_pool(name="psA_mm", bufs=2, space="PSUM") as psA_mm,
        ):
            for t in range(RT):
                ps_qt = psA_mm.tile([P, P], F32, tag="psqt")
                for dt in range(DT):
                    xsrc = x_pool.tile([P, P], F32, tag="xsrc")
                    nc.sync.dma_start(out=xsrc, in_=xin[ts(t, P), ts(dt, P)])
                    ps_x = psA_tr.tile([P, P], F32, tag="psx")
                    nc.tensor.transpose(out=ps_x, in_=xsrc, identity=ident_sb)
                    xT = xT_pool.tile([P, P], F32, tag="xT")
                    nc.scalar.copy(out=xT, in_=ps_x)
                    nc.tensor.matmul(
                        out=ps_qt,
                        lhsT=sc_t[:, ts(dt, rdim)],
                        rhs=xT,
                        start=(dt == 0),
                        stop=(dt == DT - 1),
                    )
                nc.scalar.copy(out=qt32[:, ts(t, P)], in_=ps_qt)
                nc.vector.tensor_copy(out=qtbf[:, ts(t, P)], in_=ps_qt)

        # ---- Phase B: approx scores by chunk -> packed candidates
        cand_p = cand_pool.tile([P, RT, CAND], F32, tag="cp")

        with (
            tc.tile_pool(name="ksrc", bufs=3) as ks_pool,
            tc.tile_pool(name="ktile", bufs=2) as kt_pool,
            tc.tile_pool(name="psB_tr", bufs=2, space="PSUM") as psB_tr,
            tc.tile_pool(name="psB_mm", bufs=2, space="PSUM") as psB_mm,
        ):
            for cc in range(NCH):
                ktile = kt_pool.tile([P, chunk], BF16, tag="kt")
                for j in range(KJ):
                    ksrc = ks_pool.tile([P, P], F32, tag="ks")
                    nc.sync.dma_start(
                        out=ksrc, in_=kk[ds(cc * chunk + j * P, P), :]
                    )
                    ps_k = psB_tr.tile([P, P], F32, tag="psk")
                    nc.tensor.transpose(out=ps_k, in_=ksrc, identity=ident_sb)
                    nc.scalar.copy(out=ktile[:, ts(j, P)], in_=ps_k)
                for t in range(RT):
                    ps_s = psB_mm.tile([P, chunk], F32, tag="pss")
                    for m in range(NMM):
                        nc.tensor.matmul(
                            out=ps_s[:, ts(m, 512)],
                            lhsT=qtbf[:, ts(t, P)],
                            rhs=ktile[:, ts(m, 512)],
                            start=True,
                            stop=True,
                        )
                    lt = lats[(cc * RT + t) % 2]
                    # bf16(score) -> high halfword of each u32 lattice slot
                    hi = lt[:].bitcast(BF16).rearrange(
                        "p (c two) -> p c two", two=2
                    )[:, :, 1]
                    nc.scalar.activation(out=hi, in_=ps_s[:], func=AF.Copy)
                    nc.vector.max(out=cand_p[:, t, ts(cc, 8)], in_=lt[:])

        # ---- Phase C: merge -> top16 -> exact rescore -> top8 -> output
        with (
            tc.tile_pool(name="mpool", bufs=2) as m_pool,
            tc.tile_pool(name="spool", bufs=2) as s_pool,
            tc.tile_pool(name="kcp", bufs=2) as kc_pool,
            tc.tile_pool(name="qrp", bufs=2) as qr_pool,
            tc.tile_pool(name="vgp", bufs=2) as vg_pool,
            tc.tile_pool(name="accp", bufs=2) as acc_pool,
            tc.tile_pool(name="psC_tr", bufs=2, space="PSUM") as psC_tr,
        ):
            for t in range(RT):
                cp_t = cand_p[:, t, :]
                cat16 = s_pool.tile([P, NCAND], F32, tag="cat16")
                pos16 = s_pool.tile([P, NCAND], U16, tag="pos16")
                nc.vector.max(out=cat16[:, 0:8], in_=cp_t)
                nc.vector.max_index(
                    out=pos16[:, 0:8], in_max=cat16[:, 0:8], in_values=cp_t
                )
                crep = m_pool.tile([P, CAND], F32, tag="crep")
                nc.vector.match_replace(
                    out=crep,
                    in_to_replace=cat16[:, 0:8],
                    in_values=cp_t,
                    imm_value=-1e30,
                )
                nc.vector.max(out=cat16[:, 8:16], in_=crep)
                nc.vector.max_index(
                    out=pos16[:, 8:16], in_max=cat16[:, 8:16], in_values=crep
                )
                # global idx = (packed & 0xFFFF) + (pos16//8) * chunk
                jf = s_pool.tile([P, NCAND], F32, tag="jf")
                ju = s_pool.tile([P, NCAND], U32, tag="ju")
                nc.vector.tensor_scalar(
                    ju, cat16[:].bitcast(U32), 65535, None, OP.bitwise_and
                )
                nc.vector.tensor_copy(out=jf, in_=ju)
                ccm = s_pool.tile([P, NCAND], U16, tag="ccm")
                nc.vector.tensor_scalar(
                    ccm, pos16, 0xFFF8, None, OP.bitwise_and
                )
                ccf = s_pool.tile([P, NCAND], F32, tag="ccf")
                nc.vector.tensor_copy(out=ccf, in_=ccm)
                gidx16 = s_pool.tile([P, NCAND], F32, tag="gidx16")
                nc.vector.scalar_tensor_tensor(
                    out=gidx16, in0=ccf, scalar=float(chunk / 8.0), in1=jf,
                    op0=OP.mult, op1=OP.add,
                )
                idx16 = s_pool.tile([P, NCAND], I32, tag="idx16")
                nc.vector.tensor_copy(out=idx16, in_=gidx16)

                # exact rescore of the 16 candidates
                kc = kc_pool.tile([P, NCAND, rdim], F32, tag="kc")
                for c in range(NCAND):
                    nc.gpsimd.indirect_dma_start(
                        out=kc[:, c, :],
                        out_offset=None,
                        in_=kk[:],
                        in_offset=IndirectOffsetOnAxis(
                            ap=idx16[:, c : c + 1], axis=0
                        ),
                    )
                ps_qr = psC_tr.tile([P, P], F32, tag="psqr")
                nc.tensor.transpose(
                    out=ps_qr, in_=qt32[:, ts(t, P)], identity=ident_sb
                )
                qrow = qr_pool.tile([P, P], F32, tag="qrow")
                nc.scalar.copy(out=qrow, in_=ps_qr)
                prod = kc_pool.tile([P, NCAND, rdim], F32, tag="prod")
                qr_b = (
                    qrow[:]
                    .rearrange("p (o r) -> p o r", o=1)
                    .to_broadcast([P, NCAND, rdim])
                )
                nc.vector.tensor_tensor(out=prod, in0=kc[:], in1=qr_b, op=OP.mult)
                ex16 = s_pool.tile([P, NCAND], F32, tag="ex16")
                nc.vector.tensor_reduce(out=ex16, in_=prod, axis=AX.X, op=OP.add)

                fin8 = s_pool.tile([P, 8], F32, tag="fin8")
                nc.vector.max(out=fin8, in_=ex16)
                # map fin8 values back to global indices (min idx on ties)
                bmg = s_pool.tile([P, NCAND], F32, tag="bmg")
                nc.vector.tensor_scalar(bmg, gidx16, -1.0, BIG, OP.mult, OP.add)
                eq = s_pool.tile([P, topk, NCAND], F32, tag="eq")
                ex_b = (
                    ex16[:]
                    .rearrange("p (o c) -> p o c", o=1)
                    .to_broadcast([P, topk, NCAND])
                )
                f8_b = (
                    fin8[:]
                    .rearrange("p (k o) -> p k o", o=1)
                    .to_broadcast([P, topk, NCAND])
                )
                bmg_b = (
                    bmg[:]
                    .rearrange("p (o c) -> p o c", o=1)
                    .to_broadcast([P, topk, NCAND])
                )
                nc.vector.tensor_tensor(out=eq, in0=ex_b, in1=f8_b, op=OP.is_equal)
                nc.vector.tensor_tensor(out=eq, in0=eq, in1=bmg_b, op=OP.mult)
                red = s_pool.tile([P, topk], F32, tag="red")
                nc.vector.tensor_reduce(out=red, in_=eq, axis=AX.X, op=OP.max)
                gidx8 = s_pool.tile([P, topk], F32, tag="gidx8")
                nc.vector.tensor_scalar(gidx8, red, -1.0, BIG, OP.mult, OP.add)
                idx32 = s_pool.tile([P, topk], I32, tag="idx32")
                nc.vector.tensor_copy(out=idx32, in_=gidx8)
                nc.sync.dma_start(out=out_i[ts(t, P), :], in_=idx32)

                # softmax over fin8 (sorted desc; fin8[:,0] is the max)
                nm = s_pool.tile([P, 1], F32, tag="nm")
                nc.vector.tensor_scalar(nm, fin8[:, 0:1], -1.0, None, OP.mult)
                e8 = s_pool.tile([P, topk], F32, tag="e8")
                ssum = s_pool.tile([P, 1], F32, tag="ssum")
                nc.scalar.activation(
                    out=e8, in_=fin8, func=AF.Exp, bias=nm[:], scale=1.0,
                    accum_out=ssum[:],
                )
                rcp = s_pool.tile([P, 1], F32, tag="rcp")
                nc.vector.reciprocal(rcp, ssum)
                wts = s_pool.tile([P, topk], F32, tag="wts")
                nc.vector.tensor_scalar(wts, e8, rcp[:], None, OP.mult)
                nc.sync.dma_start(out=out_w[ts(t, P), :], in_=wts)

                # gather V rows: vg[p, k, :] = kv[idx32[p, k], :]
                vg = vg_pool.tile([P, topk, d], F32, tag="vg")
                for k in range(topk):
                    nc.gpsimd.indirect_dma_start(
                        out=vg[:, k, :],
                        out_offset=None,
                        in_=kv[:],
                        in_offset=IndirectOffsetOnAxis(
                            ap=idx32[:, k : k + 1], axis=0
                        ),
                    )
                # weighted sum over the k gathered rows
                acc = acc_pool.tile([P, d], F32, tag="acc")
                nc.vector.tensor_scalar(acc, vg[:, 0, :], wts[:, 0:1], None, OP.mult)
                for k in range(1, topk):
                    nc.vector.scalar_tensor_tensor(
                        out=acc, in0=vg[:, k, :], scalar=wts[:, k : k + 1],
                        in1=acc, op0=OP.mult, op1=OP.add,
                    )
                nc.sync.dma_start(out=out_o[ts(t, P), :], in_=acc)


# ---------------------------------------------------------------------------
# Host-side entry point: full problem, 8 cores.

B, S, D = 4, 2048, 1024
N_COMPRESS = 16
N_KNOWLEDGE = 32768
RANK = 128
TOPK = 8
N_CORES = 8
ROWS_PER_CORE = (B * S) // N_CORES
CHUNK = 1024


def _build_full():
    nc = bacc.Bacc("TRN2", target_bir_lowering=False, debug=False)
    dts = {}
    dts["x"] = nc.dram_tensor("x", [ROWS_PER_CORE, D], F32, kind="ExternalInput")
    dts["mw"] = nc.dram_tensor("mw", [N_COMPRESS], F32, kind="ExternalInput")
    dts["cn"] = nc.dram_tensor(
        "cn", [N_COMPRESS, D, RANK], F32, kind="ExternalInput"
    )
    dts["kk"] = nc.dram_tensor("kk", [N_KNOWLEDGE, RANK], F32, kind="ExternalInput")
    dts["kv"] = nc.dram_tensor("kv", [N_KNOWLEDGE, D], F32, kind="ExternalInput")
    dts["ident"] = nc.dram_tensor("ident", [P, P], F32, kind="ExternalInput")
    dts["iotalat"] = nc.dram_tensor("iotalat", [CHUNK], U32, kind="ExternalInput")
    outs = {
        "out": nc.dram_tensor(
            "out", [ROWS_PER_CORE, D], F32, kind="ExternalOutput"
        ),
        "idx": nc.dram_tensor(
            "idx", [ROWS_PER_CORE, TOPK], I32, kind="ExternalOutput"
        ),
        "wts": nc.dram_tensor(
            "wts", [ROWS_PER_CORE, TOPK], F32, kind="ExternalOutput"
        ),
    }
    ins_aps = {k: v.ap() for k, v in dts.items()}
    outs_aps = {k: v.ap() for k, v in outs.items()}
    with TileContext(nc) as tc:
        build_retrieval_kernel(
            tc, outs_aps, ins_aps,
            rows=ROWS_PER_CORE, d=D, rdim=RANK, nk=N_KNOWLEDGE,
            ncomp=N_COMPRESS, topk=TOPK, chunk=CHUNK,
        )
    nc.compile()
    return nc


_NC_CACHE = None
LAST_RESULTS = None


def prepare_in_maps(x, memory_weights, compress_neurons, knowledge_K, knowledge_V):
    x = np.ascontiguousarray(np.asarray(x, dtype=np.float32)).reshape(B * S, D)
    mw = np.asarray(memory_weights, dtype=np.float32)
    cn = np.ascontiguousarray(np.asarray(compress_neurons, dtype=np.float32))
    kk = np.ascontiguousarray(np.asarray(knowledge_K, dtype=np.float32))
    kv = np.ascontiguousarray(np.asarray(knowledge_V, dtype=np.float32))

    ident = np.eye(P, dtype=np.float32)
    iotalat = np.arange(CHUNK, dtype=np.uint32)

    in_maps = []
    for c in range(N_CORES):
        in_maps.append(
            {
                "x": x[c * ROWS_PER_CORE : (c + 1) * ROWS_PER_CORE],
                "mw": mw[(c * ROWS_PER_CORE) // S],
                "cn": cn,
                "kk": kk,
                "kv": kv,
                "ident": ident,
                "iotalat": iotalat,
            }
        )
    return in_maps


def assemble_outputs(outs):
    output = np.concatenate([outs[c]["out"] for c in range(N_CORES)], axis=0)
    topk_idx = np.concatenate([outs[c]["idx"] for c in range(N_CORES)], axis=0)
    weights = np.concatenate([outs[c]["wts"] for c in range(N_CORES)], axis=0)
    output = output.reshape(B, S, D)
    topk_idx = topk_idx.reshape(B, S, TOPK).astype(np.int32)
    weights = weights.reshape(B, S, TOPK)
    return output, topk_idx, weights


def kernel(x, memory_weights, compress_neurons, knowledge_K, knowledge_V, **kwargs):
    from concourse.bass_utils import run_bass_kernel_spmd

    global _NC_CACHE
    if _NC_CACHE is None:
        _NC_CACHE = _build_full()
    nc = _NC_CACHE

    in_maps = prepare_in_maps(
        x, memory_weights, compress_neurons, knowledge_K, knowledge_V
    )
    res = run_bass_kernel_spmd(nc, in_maps, core_ids=list(range(N_CORES)), **kwargs)
    global LAST_RESULTS
    LAST_RESULTS = res
    return assemble_outputs(res.results)
